# revision 1
# baseline (speedup 1.0000x reference)
"""CapsuleLayer dynamic-routing kernel for 8 Trainium2 NeuronCores.

Strategy: data-parallel over batch (32 per core), W replicated.
Per core, hat = einsum('bie,ijed->bijd') is kept resident in SBUF (bf16,
11.8MB) in layout [p=(i%16)*8+(b%8), (c=i//16, g=b//8, j, d)]. It is
computed by PE matmuls: stationary = host-prebuilt block-diagonal input
matrices Ablk[(i',e),(i'',b'')] = x[b,i,e]*delta_{i'i''}, rhs = W chunks
[(i,e),(j,d)].

Routing (3 iters):
  s   = sum_i softmax(logits)*hat : DVE/GPSIMD bf16 mul + PE ones-blockdiag
        partition-reduce accumulated over i-chunks in PSUM.
  v   = squash(s)                 : small DVE/ACT ops.
  logits += sum_d hat*v           : DVE/GPSIMD bf16 mul + d-halving tree.
"""

import sys
from contextlib import ExitStack

import numpy as np

sys.path.insert(0, "/opt/trn_rl_repo")

import ml_dtypes  # noqa: E402

BF16 = ml_dtypes.bfloat16

B, I, E = 256, 1152, 8
J, D = 10, 16
NCORES = 8
BL = B // NCORES          # 32 batches per core
C = I // 16               # 72 i-chunks of 16
G = BL // 8               # 4 b-groups of 8
JD = J * D                # 160
GJD = G * JD              # 640
FREE = C * GJD            # 46080 free elems of hat per partition
NR = 3


def _build_kernel():
    import concourse.bass as bass
    import concourse.bacc as bacc
    import concourse.tile as tile
    from concourse import mybir

    fp32 = mybir.dt.float32
    bf16 = mybir.dt.bfloat16

    nc = bacc.Bacc("TRN2")
    t_ablk = nc.dram_tensor("ablk", [C, G, 128, 128], bf16, kind="ExternalInput")
    t_wa = nc.dram_tensor("wa", [C, 128, JD], bf16, kind="ExternalInput")
    t_inpT = nc.dram_tensor("inpT", [C, 128, BL], bf16, kind="ExternalInput")
    t_ones8 = nc.dram_tensor("ones8", [128, 8], bf16, kind="ExternalInput")
    t_biasl = nc.dram_tensor("biasl", [128, C * J], fp32, kind="ExternalInput")
    t_out = nc.dram_tensor("out", [BL, JD], fp32, kind="ExternalOutput")
    t_vd = nc.dram_tensor("vd", [BL, JD], bf16, kind="Internal")

    ap_ablk = t_ablk[:]
    ap_wa = t_wa[:]
    ap_inpT = t_inpT[:]
    ap_ones8 = t_ones8[:]
    ap_biasl = t_biasl[:]
    ap_out = t_out[:]
    ap_vd = t_vd[:]

    def bcast(ap, pos, n):
        """Insert a broadcast (step 0, count n) free dim at free-pos `pos`."""
        lst = [list(x) for x in ap.ap]
        lst.insert(1 + pos, [0, n])
        return bass.AP(tensor=ap.tensor, offset=ap.offset, ap=lst)

    def mkap(ap, dims):
        """Manual AP with explicit [step, count] dims."""
        return bass.AP(tensor=ap.tensor, offset=ap.offset,
                       ap=[list(x) for x in dims])

    with ExitStack() as ctx:
        tc = ctx.enter_context(tile.TileContext(nc))
        big = ctx.enter_context(tc.tile_pool(name="big", bufs=1))
        sing = ctx.enter_context(tc.tile_pool(name="sing", bufs=1))
        wap = ctx.enter_context(tc.tile_pool(name="wap", bufs=2))
        abp = ctx.enter_context(tc.tile_pool(name="abp", bufs=2))
        inp = ctx.enter_context(tc.tile_pool(name="inp", bufs=2))
        tmp = ctx.enter_context(tc.tile_pool(name="tmp", bufs=3))
        sfm = ctx.enter_context(tc.tile_pool(name="sfm", bufs=1))
        tre = ctx.enter_context(tc.tile_pool(name="tre", bufs=2))
        sml = ctx.enter_context(tc.tile_pool(name="sml", bufs=1))
        psA = ctx.enter_context(tc.tile_pool(name="psA", bufs=5, space="PSUM"))
        psS = ctx.enter_context(tc.tile_pool(name="psS", bufs=1, space="PSUM"))

        hat = big.tile([128, FREE], bf16)
        logits = sing.tile([128, C * G * J], fp32)
        ones8 = sing.tile([128, 8], bf16)
        v_rep = sing.tile([128, GJD], bf16)
        biasl = sing.tile([128, C * J], fp32)
        nc.sync.dma_start(out=ones8, in_=ap_ones8)
        nc.sync.dma_start(out=biasl, in_=ap_biasl)

        # ---------------- Phase A: hat + s0 ----------------
        ps0 = psS.tile([BL, JD], fp32)
        ev = [0]

        def evac(ps, lo_cg, n_cg):
            # copy psum [128, n_cg*160] -> hat slice, alternating ACT/DVE
            dst = hat[:, lo_cg * JD:(lo_cg + n_cg) * JD]
            src = ps[:, : n_cg * JD]
            if ev[0] % 2 == 0:
                nc.scalar.copy(dst, src)
            else:
                nc.vector.tensor_copy(dst, src)
            ev[0] += 1

        ps = None
        SLAB = 9  # c-chunks per DMA slab
        for sl in range(C // SLAB):
            c0 = sl * SLAB
            wa_s = wap.tile([128, SLAB * JD], bf16)
            nc.sync.dma_start(
                out=wa_s.rearrange("p (c f) -> p c f", c=SLAB),
                in_=ap_wa[c0:c0 + SLAB].rearrange("c p f -> p c f"))
            in_s = inp.tile([128, SLAB * BL], bf16)
            nc.sync.dma_start(
                out=in_s.rearrange("p (c f) -> p c f", c=SLAB),
                in_=ap_inpT[c0:c0 + SLAB].rearrange("c p f -> p c f"))
            ab_s = abp.tile([128, SLAB * G * 128], bf16)
            nc.sync.dma_start(
                out=ab_s.rearrange("p (c g f) -> p c g f", c=SLAB, g=G),
                in_=ap_ablk[c0:c0 + SLAB].rearrange("c g p f -> p c g f"))
            for cc in range(SLAB):
                c = c0 + cc
                wa_t = wa_s[:, cc * JD:(cc + 1) * JD]
                nc.tensor.matmul(ps0, in_s[:, cc * BL:(cc + 1) * BL], wa_t,
                                 start=(c == 0), stop=(c == C - 1))
                for g in range(G):
                    k = c * G + g
                    slot = k % 3
                    if slot == 0:
                        ps = psA.tile([128, 3 * JD], fp32)
                    nc.tensor.matmul(
                        ps[:, slot * JD:(slot + 1) * JD],
                        ab_s[:, (cc * G + g) * 128:(cc * G + g + 1) * 128],
                        wa_t, start=True, stop=True)
                    if slot == 2:
                        evac(ps, k - 2, 3)
        # tail (C*G = 288 divisible by 3 -> no tail)

        # ---------------- helpers ----------------
        def squash_and_vrep(s_sb, P, nj, first):
            """s_sb: [P, nj*16] f32 view (nj j-like groups). Returns v f32."""
            sq = sml.tile([P, nj * D], fp32, tag="sq")
            nc.vector.tensor_mul(sq, s_sb, s_sb)
            s2 = sml.tile([P, nj], fp32, tag="s2")
            nc.vector.tensor_reduce(
                s2, sq.rearrange("p (j d) -> p j d", d=D),
                axis=mybir.AxisListType.X, op=mybir.AluOpType.add)
            rt = sml.tile([P, nj], fp32, tag="rt")
            nc.scalar.sqrt(rt, s2)
            den = sml.tile([P, nj], fp32, tag="den")
            nc.vector.scalar_tensor_tensor(
                out=den, in0=s2, scalar=1.0, in1=rt,
                op0=mybir.AluOpType.add, op1=mybir.AluOpType.mult)
            rden = sml.tile([P, nj], fp32, tag="rden")
            nc.vector.reciprocal(rden, den)
            sc = sml.tile([P, nj], fp32, tag="sc")
            nc.vector.tensor_mul(sc, s2, rden)
            v_f = sml.tile([P, nj * D], fp32, tag="v_f")
            sc3 = bcast(sc, 1, D)  # [P, nj, D(bcast)]
            nc.vector.tensor_tensor(
                out=v_f.rearrange("p (j d) -> p j d", d=D),
                in0=s_sb.rearrange("p (j d) -> p j d", d=D),
                in1=sc3, op=mybir.AluOpType.mult)
            return v_f

        def fill_vrep(v_bf, P):
            # v_bf [P, x] with P=8 (x=GJD) direct; P=BL via dram roundtrip
            if P == 8:
                for i_ in range(16):
                    nc.gpsimd.dma_start(out=v_rep[8 * i_:8 * (i_ + 1), :], in_=v_bf)
            else:
                nc.gpsimd.dma_start(out=ap_vd, in_=v_bf)
                # vd[32,160] viewed as [b'':8, g:4, f:160]
                src = mkap(ap_vd, [[JD, 8], [8 * JD, G], [1, JD]])
                for i_ in range(16):
                    dst = v_rep[8 * i_:8 * (i_ + 1), :].rearrange(
                        "b (g f) -> b g f", g=G)
                    nc.gpsimd.dma_start(out=dst, in_=src)

        NCB = 12            # c-chunks for elementwise passes
        CC = C // NCB       # 6 c per chunk

        def agreement(first):
            """logits (+)= sum_d hat * v_rep."""
            for cb in range(NCB):
                eng = nc.gpsimd if cb % 3 == 2 else nc.vector
                lo = cb * CC * GJD
                p2 = tmp.tile([128, CC * GJD], bf16, tag="p2")
                vin = bcast(v_rep[:, :], 0, CC)  # [128, CC, GJD(strided)]
                eng.tensor_tensor(
                    out=p2.rearrange("p (c f) -> p c f", c=CC),
                    in0=hat[:, lo:lo + CC * GJD].rearrange(
                        "p (c f) -> p c f", c=CC),
                    in1=vin, op=mybir.AluOpType.mult)
                n = CC * G * J
                t1 = tre.tile([128, n * 8], bf16, tag="t1")
                p2v = p2.rearrange("p (n d) -> p n d", d=D)
                t1v = t1.rearrange("p (n d) -> p n d", d=8)
                eng.tensor_tensor(out=t1v, in0=p2v[:, :, 0:8],
                                  in1=p2v[:, :, 8:16], op=mybir.AluOpType.add)
                t2 = tre.tile([128, n * 4], bf16, tag="t2")
                t2v = t2.rearrange("p (n d) -> p n d", d=4)
                eng.tensor_tensor(out=t2v, in0=t1v[:, :, 0:4],
                                  in1=t1v[:, :, 4:8], op=mybir.AluOpType.add)
                t3 = tre.tile([128, n * 2], bf16, tag="t3")
                t3v = t3.rearrange("p (n d) -> p n d", d=2)
                eng.tensor_tensor(out=t3v, in0=t2v[:, :, 0:2],
                                  in1=t2v[:, :, 2:4], op=mybir.AluOpType.add)
                t4 = tre.tile([128, n], fp32, tag="t4")
                eng.tensor_tensor(out=t4, in0=t3v[:, :, 0],
                                  in1=t3v[:, :, 1], op=mybir.AluOpType.add)
                lsl = logits[:, cb * n:(cb + 1) * n]
                if first:
                    # logits = t4 + bias (bias bcast over g)
                    bsl = biasl[:, cb * CC * J:(cb + 1) * CC * J]
                    eng.tensor_tensor(
                        out=lsl.rearrange("p (c g j) -> p c g j", c=CC, g=G),
                        in0=t4.rearrange("p (c g j) -> p c g j", c=CC, g=G),
                        in1=bcast(bsl.rearrange("p (c j) -> p c j", c=CC), 1, G),
                        op=mybir.AluOpType.add)
                else:
                    eng.tensor_tensor(out=lsl, in0=lsl, in1=t4,
                                      op=mybir.AluOpType.add)

        def softmax_c():
            ex = sfm.tile([128, C * G * J], bf16, tag="ex")
            nc.scalar.activation(ex, logits, mybir.ActivationFunctionType.Exp)
            se = sml.tile([128, C * G], fp32, tag="se")
            nc.vector.tensor_reduce(
                se, ex.rearrange("p (n j) -> p n j", j=J),
                axis=mybir.AxisListType.X, op=mybir.AluOpType.add)
            rse = sml.tile([128, C * G], bf16, tag="rse")
            with nc.allow_low_precision(reason="softmax denom bf16 is enough"):
                nc.vector.reciprocal(rse, se)
            c_t = sfm.tile([128, C * G * J], bf16, tag="c_t")
            nc.vector.tensor_tensor(
                out=c_t.rearrange("p (n j) -> p n j", j=J),
                in0=ex.rearrange("p (n j) -> p n j", j=J),
                in1=bcast(rse[:, :], 1, J), op=mybir.AluOpType.mult)
            return c_t

        def s_phase(c_t):
            """returns s psum tiles ([8,320] x2) = sum_i c*hat."""
            pa = psS.tile([8, GJD // 2], fp32, tag="pa")
            pb = psS.tile([8, GJD // 2], fp32, tag="pb")
            for cb in range(NCB):
                eng = nc.gpsimd if cb % 3 == 2 else nc.vector
                lo = cb * CC * GJD
                p_ = tmp.tile([128, CC * GJD], bf16, tag="p2")
                csl = c_t[:, cb * CC * G * J:(cb + 1) * CC * G * J]
                eng.tensor_tensor(
                    out=p_.rearrange("p (n d) -> p n d", d=D),
                    in0=hat[:, lo:lo + CC * GJD].rearrange(
                        "p (n d) -> p n d", d=D),
                    in1=bcast(csl, 1, D), op=mybir.AluOpType.mult)
                for ci in range(CC):
                    k = cb * CC + ci
                    h = GJD // 2
                    nc.tensor.matmul(pa, ones8, p_[:, ci * GJD:ci * GJD + h],
                                     start=(k == 0), stop=(k == C - 1))
                    nc.tensor.matmul(pb, ones8, p_[:, ci * GJD + h:(ci + 1) * GJD],
                                     start=(k == 0), stop=(k == C - 1))
            return pa, pb

        # ---------------- iteration 0 ----------------
        s0 = sml.tile([BL, JD], fp32, tag="s0")
        nc.scalar.mul(s0, ps0, 1.0 / J)
        v0 = squash_and_vrep(s0, BL, J, True)
        v0b = sml.tile([BL, JD], bf16, tag="v0b")
        nc.vector.tensor_copy(v0b, v0)
        fill_vrep(v0b, BL)
        agreement(first=True)

        # ---------------- iterations 1..2 ----------------
        for r in range(1, NR):
            c_t = softmax_c()
            pa, pb = s_phase(c_t)
            s_sb = sml.tile([8, GJD], fp32, tag="s_sb")
            nc.scalar.copy(s_sb[:, :GJD // 2], pa)
            nc.scalar.copy(s_sb[:, GJD // 2:], pb)
            v = squash_and_vrep(s_sb, 8, G * J, False)
            if r == NR - 1:
                dst = mkap(ap_out, [[JD, 8], [8 * JD, G], [1, JD]])
                nc.gpsimd.dma_start(
                    out=dst, in_=v.rearrange("p (g f) -> p g f", g=G))
            else:
                vb = sml.tile([8, GJD], bf16, tag="vb")
                nc.vector.tensor_copy(vb, v)
                fill_vrep(vb, 8)
                agreement(first=False)

    nc.finalize()
    return nc


def _host_prep(inputs, W, bias):
    """Build per-core input maps."""
    W = np.asarray(inputs["W"] if isinstance(inputs, dict) else W)
    x = np.asarray(inputs["inputs"] if isinstance(inputs, dict) else inputs)
    wa = np.ascontiguousarray(
        W.transpose(0, 2, 1, 3).reshape(C, 16 * E, JD)).astype(BF16)
    ones8 = np.zeros((128, 8), BF16)
    ones8[np.arange(128), np.arange(128) % 8] = 1
    b2 = np.asarray(bias).reshape(I, J).astype(np.float32)
    br = b2.reshape(C, 16, J).transpose(1, 0, 2)        # [i'',c,j]
    biasl = np.ascontiguousarray(
        np.broadcast_to(br[:, None], (16, 8, C, J)).reshape(128, C * J))
    maps = []
    for cl in range(NCORES):
        xl = x[cl * BL:(cl + 1) * BL]                   # [32,1152,8]
        inpT = np.ascontiguousarray(
            xl.transpose(1, 2, 0).reshape(C, 128, BL)).astype(BF16)
        xr = xl.reshape(G, 8, C, 16, E).transpose(2, 0, 3, 4, 1)  # [c,g,i,e,b]
        A6 = np.zeros((C, G, 16, E, 16, 8), np.float32)
        for i_ in range(16):
            A6[:, :, i_, :, i_, :] = xr[:, :, i_, :, :]
        ablk = A6.reshape(C, G, 128, 128).astype(BF16)
        maps.append({"ablk": ablk, "wa": wa, "inpT": inpT,
                     "ones8": ones8, "biasl": biasl})
    return maps


_NC_CACHE = {}


def kernel(inputs, W, bias):
    from concourse import bass_utils

    if "nc" not in _NC_CACHE:
        _NC_CACHE["nc"] = _build_kernel()
    nc = _NC_CACHE["nc"]
    in_maps = _host_prep({"inputs": inputs, "W": W}, W, bias)
    res = bass_utils.run_bass_kernel_spmd(nc, in_maps, core_ids=list(range(NCORES)))
    out = np.concatenate(
        [r["out"].reshape(BL, J, D) for r in res.results], axis=0)
    return out.astype(np.float32)


if __name__ == "__main__":
    import reference
    ins = reference.setup_inputs()
    ins = {k: np.asarray(v) for k, v in ins.items()}
    exp = np.asarray(reference.reference(**ins))
    got = kernel(**ins)
    err = np.abs(got - exp).max() / (np.abs(exp).max() + 1e-9)
    print("Relative error:", err)



# revision 9
# speedup vs baseline: 1.5688x; 1.5688x over previous
"""CapsuleLayer dynamic-routing kernel for 8 Trainium2 NeuronCores.

Data-parallel over batch (32 per core), W replicated.

Per-core layout: partitions p = i4*32 + b  (i4 = i mod 4, b = 0..31),
hat[p, (c, d, j)] bf16 resident in SBUF, c = i//4 (288 chunks), d=16, j=10
(j innermost so every big elementwise op keeps DVE 2x mode: all access
patterns are 2-byte with innermost step 1).

Phase A (hat): per c, matmul k=(i4',e)=32: stationary = host-built
block-diag x (ab[32,128]), moving = W slice wa[32,160] -> psum [128,160],
evacuated to hat (ACT/DVE/Pool rotate). s0 = sum_i c0*hat is accumulated
by 72 full-k matmuls: stationary xt[128,32] (x transposed, k=(cs,i4,e)),
moving wc[128,160] (W pre-scaled by c0 = softmax_j(bias) host-side).

Routing iters: agreement a = sum_d hat*v via DVE mul + d-halving tree
(bf16, all 2x); softmax over innermost j; s = sum_i c*hat via DVE mul +
PE ones-blockdiag matmuls accumulating [32,160] psum over 288 c-chunks.
GPSIMD (Pool) takes a ~21% slice of each big elementwise pass.
"""

import sys
from contextlib import ExitStack

import numpy as np

sys.path.insert(0, "/opt/trn_rl_repo")

import ml_dtypes  # noqa: E402

BF16 = ml_dtypes.bfloat16

B, I, E = 256, 1152, 8
J, D = 10, 16
NCORES = 8
BL = B // NCORES          # 32 batches per core
C = I // 4                # 288 i-chunks of 4
G = C // 4                # 72 groups of 4 c-chunks (16 i) for s0 matmuls
DJ = D * J                # 160
FREE = C * DJ             # 46080 free elems of hat per partition
NR = 3

POOL_C = 60               # c-chunks handled by GPSIMD per big pass
POOL_CHUNK = 30
DVE_C = C - POOL_C        # 228
DVE_CHUNK = 38            # 6 chunks
SLAB_G = 4                # groups per DMA slab (18 slabs)


def _build_kernel():
    import concourse.bass as bass
    import concourse.bacc as bacc
    import concourse.tile as tile
    from concourse import mybir

    fp32 = mybir.dt.float32
    bf16 = mybir.dt.bfloat16

    nc = bacc.Bacc("TRN2")
    t_ab = nc.dram_tensor("ab", [32, C, 128], bf16, kind="ExternalInput")
    t_wa = nc.dram_tensor("wa", [32, C, DJ], bf16, kind="ExternalInput")
    t_wc = nc.dram_tensor("wc", [128, G, DJ], bf16, kind="ExternalInput")
    t_xt = nc.dram_tensor("xt", [128, G, BL], bf16, kind="ExternalInput")
    t_ones = nc.dram_tensor("ones", [128, BL], bf16, kind="ExternalInput")
    t_bias = nc.dram_tensor("biasl", [128, C * J], bf16, kind="ExternalInput")
    t_out = nc.dram_tensor("out", [BL, DJ], fp32, kind="ExternalOutput")

    def bcast(ap, pos, n):
        """Insert a broadcast (step 0, count n) free dim at free-pos pos."""
        lst = [list(x) for x in ap.ap]
        lst.insert(1 + pos, [0, n])
        return bass.AP(tensor=ap.tensor, offset=ap.offset, ap=lst)

    with ExitStack() as ctx:
        tc = ctx.enter_context(tile.TileContext(nc))
        big = ctx.enter_context(tc.tile_pool(name="big", bufs=1))
        sing = ctx.enter_context(tc.tile_pool(name="sing", bufs=1))
        abp = ctx.enter_context(tc.tile_pool(name="abp", bufs=2))
        wap = ctx.enter_context(tc.tile_pool(name="wap", bufs=2))
        wcp = ctx.enter_context(tc.tile_pool(name="wcp", bufs=2))
        xtp = ctx.enter_context(tc.tile_pool(name="xtp", bufs=2))
        tmp = ctx.enter_context(tc.tile_pool(name="tmp", bufs=3))
        tre = ctx.enter_context(tc.tile_pool(name="tre", bufs=2))
        sfm = ctx.enter_context(tc.tile_pool(name="sfm", bufs=1))
        sml = ctx.enter_context(tc.tile_pool(name="sml", bufs=1))
        psA = ctx.enter_context(tc.tile_pool(name="psA", bufs=4, space="PSUM"))
        ps0p = ctx.enter_context(tc.tile_pool(name="ps0p", bufs=1, space="PSUM"))
        psS = ctx.enter_context(tc.tile_pool(name="psS", bufs=2, space="PSUM"))

        hat = big.tile([128, FREE], bf16)
        logits = sing.tile([128, C * J], fp32)
        onesb = sing.tile([128, BL], bf16)
        biasl = sing.tile([128, C * J], bf16)
        v_rep = sing.tile([128, DJ], bf16)
        nc.sync.dma_start(out=onesb, in_=t_ones[:])
        nc.sync.dma_start(out=biasl, in_=t_bias[:])

        # ---------------- Phase A: hat + s0 ----------------
        ps0 = ps0p.tile([BL, DJ], fp32)
        ev = [0]

        def evac(ps, lo_c, n_c):
            dst = hat[:, lo_c * DJ:(lo_c + n_c) * DJ]
            src = ps[:, :n_c * DJ]
            # GPSIMD cannot access PSUM; rotate ACT/DVE (ACT slightly cheaper)
            k = ev[0] % 15
            if k % 2 == 0 or k == 13:
                nc.scalar.copy(dst, src)
            else:
                nc.vector.tensor_copy(dst, src)
            ev[0] += 1

        ps = None
        SLAB_C = SLAB_G * 4
        for sl in range(G // SLAB_G):
            g0 = sl * SLAB_G
            c0 = g0 * 4
            ab_s = abp.tile([32, SLAB_C * 128], bf16)
            nc.sync.dma_start(
                out=ab_s.rearrange("p (c f) -> p c f", c=SLAB_C),
                in_=t_ab[:, c0:c0 + SLAB_C])
            wa_s = wap.tile([32, SLAB_C * DJ], bf16)
            nc.sync.dma_start(
                out=wa_s.rearrange("p (c f) -> p c f", c=SLAB_C),
                in_=t_wa[:, c0:c0 + SLAB_C])
            wc_s = wcp.tile([128, SLAB_G * DJ], bf16)
            nc.sync.dma_start(
                out=wc_s.rearrange("p (g f) -> p g f", g=SLAB_G),
                in_=t_wc[:, g0:g0 + SLAB_G])
            xt_s = xtp.tile([128, SLAB_G * BL], bf16)
            nc.sync.dma_start(
                out=xt_s.rearrange("p (g f) -> p g f", g=SLAB_G),
                in_=t_xt[:, g0:g0 + SLAB_G])
            for gl in range(SLAB_G):
                g = g0 + gl
                for cs in range(4):
                    cl = gl * 4 + cs
                    cc = g * 4 + cs
                    slot = cc % 3
                    if slot == 0:
                        ps = psA.tile([128, 3 * DJ], fp32)
                    nc.tensor.matmul(
                        ps[:, slot * DJ:(slot + 1) * DJ],
                        ab_s[:, cl * 128:(cl + 1) * 128],
                        wa_s[:, cl * DJ:(cl + 1) * DJ],
                        start=True, stop=True)
                    if slot == 2:
                        evac(ps, cc - 2, 3)
                nc.tensor.matmul(
                    ps0, xt_s[:, gl * BL:(gl + 1) * BL],
                    wc_s[:, gl * DJ:(gl + 1) * DJ],
                    start=(g == 0), stop=(g == G - 1))
        # C = 288 divisible by 3 -> no evac tail

        # ---------------- helpers ----------------
        def squash(s_ap):
            """s_ap: [BL, (d j)] fp32 (may be PSUM). Returns v fp32 [BL, DJ]."""
            s_sb = sml.tile([BL, DJ], fp32, tag="s_sb")
            nc.scalar.copy(s_sb, s_ap)
            s_ap = s_sb
            sq = sml.tile([BL, DJ], fp32, tag="sq")
            nc.vector.tensor_mul(sq, s_ap, s_ap)
            sqv = sq.rearrange("p (d j) -> p d j", d=D)
            q1 = sml.tile([BL, 8 * J], fp32, tag="q1")
            q1v = q1.rearrange("p (d j) -> p d j", d=8)
            nc.vector.tensor_tensor(out=q1v, in0=sqv[:, 0:8], in1=sqv[:, 8:16],
                                    op=mybir.AluOpType.add)
            q2 = sml.tile([BL, 4 * J], fp32, tag="q2")
            q2v = q2.rearrange("p (d j) -> p d j", d=4)
            nc.vector.tensor_tensor(out=q2v, in0=q1v[:, 0:4], in1=q1v[:, 4:8],
                                    op=mybir.AluOpType.add)
            q3 = sml.tile([BL, 2 * J], fp32, tag="q3")
            q3v = q3.rearrange("p (d j) -> p d j", d=2)
            nc.vector.tensor_tensor(out=q3v, in0=q2v[:, 0:2], in1=q2v[:, 2:4],
                                    op=mybir.AluOpType.add)
            s2 = sml.tile([BL, J], fp32, tag="s2")
            nc.vector.tensor_tensor(out=s2, in0=q3v[:, 0], in1=q3v[:, 1],
                                    op=mybir.AluOpType.add)
            rt = sml.tile([BL, J], fp32, tag="rt")
            nc.scalar.sqrt(rt, s2)
            den = sml.tile([BL, J], fp32, tag="den")
            nc.vector.scalar_tensor_tensor(
                out=den, in0=s2, scalar=1.0, in1=rt,
                op0=mybir.AluOpType.add, op1=mybir.AluOpType.mult)
            rden = sml.tile([BL, J], fp32, tag="rden")
            nc.vector.reciprocal(rden, den)
            sc = sml.tile([BL, J], fp32, tag="sc")
            nc.vector.tensor_mul(sc, s2, rden)
            v_f = sml.tile([BL, DJ], fp32, tag="v_f")
            nc.vector.tensor_tensor(
                out=v_f.rearrange("p (d j) -> p d j", d=D),
                in0=s_ap.rearrange("p (d j) -> p d j", d=D),
                in1=bcast(sc[:, :], 0, D),
                op=mybir.AluOpType.mult)
            return v_f

        def fill_vrep(v_f):
            vb = sml.tile([BL, DJ], bf16, tag="vb")
            nc.vector.tensor_copy(vb, v_f)
            nc.vector.tensor_copy(v_rep[0:BL, :], vb)
            nc.vector.tensor_copy(v_rep[BL:2 * BL, :], v_rep[0:BL, :])
            nc.vector.tensor_copy(v_rep[2 * BL:4 * BL, :], v_rep[0:2 * BL, :])

        def agree_chunk(eng, lo, cn, first):
            """logits[:, lo*J:(lo+cn)*J] (+)= sum_d hat*v_rep over c-range."""
            p1 = tmp.tile([128, cn * DJ], bf16, tag=f"p{eng is nc.gpsimd}",
                          bufs=2)
            eng.tensor_tensor(
                out=p1.rearrange("p (c f) -> p c f", c=cn),
                in0=hat[:, lo * DJ:(lo + cn) * DJ].rearrange(
                    "p (c f) -> p c f", c=cn),
                in1=bcast(v_rep[:, :], 0, cn),
                op=mybir.AluOpType.mult)
            p1v = p1.rearrange("p (c d j) -> p c d j", c=cn, d=D)
            t1 = tre.tile([128, cn * 8 * J], bf16, tag=f"t1{eng is nc.gpsimd}", bufs=1)
            t1v = t1.rearrange("p (c d j) -> p c d j", c=cn, d=8)
            eng.tensor_tensor(out=t1v, in0=p1v[:, :, 0:8], in1=p1v[:, :, 8:16],
                              op=mybir.AluOpType.add)
            t2 = tre.tile([128, cn * 4 * J], bf16, tag=f"t2{eng is nc.gpsimd}", bufs=1)
            t2v = t2.rearrange("p (c d j) -> p c d j", c=cn, d=4)
            eng.tensor_tensor(out=t2v, in0=t1v[:, :, 0:4], in1=t1v[:, :, 4:8],
                              op=mybir.AluOpType.add)
            t3 = tre.tile([128, cn * 2 * J], bf16, tag=f"t3{eng is nc.gpsimd}", bufs=1)
            t3v = t3.rearrange("p (c d j) -> p c d j", c=cn, d=2)
            eng.tensor_tensor(out=t3v, in0=t2v[:, :, 0:2], in1=t2v[:, :, 2:4],
                              op=mybir.AluOpType.add)
            t4 = tre.tile([128, cn * J], bf16, tag=f"t4{eng is nc.gpsimd}", bufs=1)
            t4v = t4.rearrange("p (c j) -> p c j", c=cn)
            eng.tensor_tensor(out=t4v, in0=t3v[:, :, 0], in1=t3v[:, :, 1],
                              op=mybir.AluOpType.add)
            lsl = logits[:, lo * J:(lo + cn) * J]
            if first:
                eng.tensor_tensor(out=lsl, in0=t4,
                                  in1=biasl[:, lo * J:(lo + cn) * J],
                                  op=mybir.AluOpType.add)
            else:
                eng.tensor_tensor(out=lsl, in0=lsl, in1=t4,
                                  op=mybir.AluOpType.add)

        def agreement(first):
            for pk in range(POOL_C // POOL_CHUNK):
                lo = DVE_C + pk * POOL_CHUNK
                agree_chunk(nc.gpsimd, lo, POOL_CHUNK, first)
            for dk in range(DVE_C // DVE_CHUNK):
                agree_chunk(nc.vector, dk * DVE_CHUNK, DVE_CHUNK, first)

        def softmax_c():
            ex = sfm.tile([128, C * J], bf16, tag="ex")
            nc.scalar.activation(ex, logits, mybir.ActivationFunctionType.Exp)
            se = sml.tile([128, C], fp32, tag="se")
            nc.vector.tensor_reduce(
                se, ex.rearrange("p (c j) -> p c j", j=J),
                axis=mybir.AxisListType.X, op=mybir.AluOpType.add)
            rse = sml.tile([128, C], bf16, tag="rse")
            with nc.allow_low_precision(reason="softmax denom bf16 is enough"):
                nc.vector.reciprocal(rse, se)
            nc.vector.tensor_tensor(
                out=ex.rearrange("p (c j) -> p c j", j=J),
                in0=ex.rearrange("p (c j) -> p c j", j=J),
                in1=bcast(rse[:, :], 1, J), op=mybir.AluOpType.mult)
            return ex

        def s_chunk(eng, c_t, lo, cn):
            p2 = tmp.tile([128, cn * DJ], bf16, tag=f"p{eng is nc.gpsimd}",
                          bufs=2)
            cin = bcast(c_t[:, lo * J:(lo + cn) * J].rearrange(
                "p (c j) -> p c j", j=J), 1, D)   # [p, c, D(bc), j]
            eng.tensor_tensor(
                out=p2.rearrange("p (c d j) -> p c d j", c=cn, d=D),
                in0=hat[:, lo * DJ:(lo + cn) * DJ].rearrange(
                    "p (c d j) -> p c d j", c=cn, d=D),
                in1=cin, op=mybir.AluOpType.mult)
            return p2

        def s_phase(c_t):
            """returns psum [BL, DJ] = sum_i c*hat."""
            pss = psS.tile([BL, DJ], fp32, tag="pss")
            seq = [0]

            def reduce_mm(p2, lo, cn):
                for ci in range(cn):
                    k = seq[0]
                    nc.tensor.matmul(
                        pss, onesb, p2[:, ci * DJ:(ci + 1) * DJ],
                        start=(k == 0), stop=(k == C - 1))
                    seq[0] += 1

            pool_p2 = []
            for pk in range(POOL_C // POOL_CHUNK):
                lo = DVE_C + pk * POOL_CHUNK
                pool_p2.append((s_chunk(nc.gpsimd, c_t, lo, POOL_CHUNK),
                                lo, POOL_CHUNK))
            for dk in range(DVE_C // DVE_CHUNK):
                lo = dk * DVE_CHUNK
                p2 = s_chunk(nc.vector, c_t, lo, DVE_CHUNK)
                reduce_mm(p2, lo, DVE_CHUNK)
            for p2, lo, cn in pool_p2:
                reduce_mm(p2, lo, cn)
            return pss

        # ---------------- iteration 0 ----------------
        v0 = squash(ps0)
        fill_vrep(v0)
        agreement(first=True)

        # ---------------- iterations 1..2 ----------------
        for r in range(1, NR):
            c_t = softmax_c()
            pss = s_phase(c_t)
            v = squash(pss)
            if r == NR - 1:
                nc.sync.dma_start(out=t_out[:], in_=v)
            else:
                fill_vrep(v)
                agreement(first=False)

    nc.finalize()
    return nc


_PREP = {}


def _host_prep(x, W, bias):
    """Build per-core input maps. x:[256,1152,8] W:[1152,10,8,16] bias:[1,1152,10]"""
    x = np.asarray(x, np.float32)
    W = np.asarray(W, np.float32)
    bias = np.asarray(bias, np.float32).reshape(I, J)

    # W arranged [k=(i4,e)=32, c, (d,j)]
    Wr = W.reshape(C, 4, J, E, D)                     # [c, i4, j, e, d]
    wa = np.ascontiguousarray(
        Wr.transpose(1, 3, 0, 4, 2).reshape(32, C, DJ)).astype(BF16)

    # c0 = softmax_j(bias) folded into a W copy for the s0 accumulation
    eb = np.exp(bias - bias.max(axis=1, keepdims=True))
    c0 = (eb / eb.sum(axis=1, keepdims=True)).astype(np.float32)  # [I, J]
    Wc = W * c0[:, :, None, None]
    Wcr = Wc.reshape(G, 4, 4, J, E, D)
    wc = np.ascontiguousarray(
        Wcr.transpose(1, 2, 4, 0, 5, 3).reshape(128, G, DJ)).astype(BF16)

    onesm = np.tile(np.eye(BL, dtype=np.float32), (4, 1)).astype(BF16)

    b3 = bias.reshape(C, 4, J)                        # [c, i4, j]
    biasl = np.ascontiguousarray(np.broadcast_to(
        b3.transpose(1, 0, 2).reshape(4, 1, C * J),
        (4, BL, C * J)).reshape(128, C * J)).astype(BF16)

    maps = []
    for cl in range(NCORES):
        xl = x[cl * BL:(cl + 1) * BL]                 # [32, 1152, 8]
        xr = xl.reshape(BL, G, 4, 4, E)               # [b, grp, cs, i4, e]
        xc = xl.reshape(BL, C, 4, E)                  # [b, c, i4, e]
        ab6 = np.zeros((4, E, C, 4, BL), np.float32)
        for ip in range(4):
            # [e, c, b]
            ab6[ip, :, :, ip, :] = xc[:, :, ip, :].transpose(2, 1, 0)
        ab = ab6.reshape(32, C, 128).astype(BF16)
        xt = np.ascontiguousarray(
            xr.transpose(2, 3, 4, 1, 0).reshape(128, G, BL)).astype(BF16)
        maps.append({"ab": ab, "wa": wa, "wc": wc, "xt": xt,
                     "ones": onesm, "biasl": biasl})
    return maps


_NC_CACHE = {}


def kernel(inputs, W, bias):
    from concourse import bass_utils

    if "nc" not in _NC_CACHE:
        _NC_CACHE["nc"] = _build_kernel()
    nc = _NC_CACHE["nc"]
    in_maps = _host_prep(inputs, W, bias)
    res = bass_utils.run_bass_kernel_spmd(nc, in_maps, core_ids=list(range(NCORES)))
    # out is [32, (d,j)] per core -> [32, J, D]
    out = np.concatenate(
        [r["out"].reshape(BL, D, J).transpose(0, 2, 1) for r in res.results],
        axis=0)
    return np.ascontiguousarray(out).astype(np.float32)


if __name__ == "__main__":
    import reference
    ins = reference.setup_inputs()
    ins = {k: np.asarray(v) for k, v in ins.items()}
    exp = np.asarray(reference.reference(**ins))
    got = kernel(**ins)
    err = np.abs(got - exp).max() / (np.abs(exp).max() + 1e-9)
    print("Relative error:", err)
